# revision 11
# baseline (speedup 1.0000x reference)
"""CapsuleLayer dynamic-routing kernel for Trainium2 (8 NeuronCores).

Problem: inputs [B=32, I=2048, J=16], W [N=64, I=2048, D=32, J=16], routings=3.
  inputs_hat[b,n,i,d] = sum_j inputs[b,i,j] * W[n,i,d,j]
  3 rounds of routing (softmax over n, weighted sum over i, squash over d).

Strategy: shard the input-capsule axis I across the 8 cores (I_loc=256).
Each core recomputes its ihat shard from W each round (W streamed from HBM,
ihat never materialized to DRAM), keeps its b-state [*, n, i_loc] in SBUF,
and the only cross-core data is the [B, N, D] partial sum s, AllReduced
(256 KB) once per round. Output is replicated; host takes core 0's.

On-chip layout per round, per group of 4 input capsules i:
  PE: 4 col-tiled matmuls (tile_position=(0,32c)), K=j=16, M=b=32,
      Nf=(n,d)=2048 -> PSUM H-tile [128=(4i,32b), 2048=(64n,32d)]
  DVE/ACT: y = sum_d H*v ; b += y ; c = softmax_n(b) ; s_acc += c*H
"""

import sys

for p in ("/opt/trn_rl_repo",):
    if p not in sys.path:
        sys.path.insert(0, p)

import numpy as np

import concourse.bacc as bacc
import concourse.bass as bass
import concourse.mybir as mybir
import concourse.tile as tile
from concourse.bass_utils import run_bass_kernel_spmd

# problem constants (hardcoded per harness contract)
B, N, I, D, J = 32, 64, 2048, 32, 16
R = 3  # routings
CORES = 8
I_LOC = I // CORES  # 256
ND = N * D  # 2048
EPS = 1e-7

F32 = mybir.dt.float32
FX = mybir.AxisListType.X
ADD = mybir.AluOpType.add
MAX = mybir.AluOpType.max
ACT = mybir.ActivationFunctionType

GROUPS = I_LOC // 4  # 64 groups of 4 capsules per round


def _squash_build(nc, pool, vbpool, smalls, s4, eps_ap):
    """s4: [128, 2048] tile holding s (replicated x4 on partition groups).
    Returns vb [128, 2048] = squash(s) broadcast tile (same replication)."""
    s2 = pool.tile([128, ND], F32, tag="tmp")
    nc.vector.tensor_mul(s2[:], s4[:], s4[:])
    sq = smalls.tile([128, N], F32, tag="sq_sq")
    nc.vector.tensor_reduce(sq[:], s2[:].rearrange("p (n d) -> p n d", d=D),
                            axis=FX, op=ADD)
    # t = sqrt(sq + eps)
    t = smalls.tile([128, N], F32, tag="sq_t")
    nc.scalar.activation(t[:], sq[:], ACT.Sqrt, bias=eps_ap)
    # q1 = 1 + sq
    q1 = smalls.tile([128, N], F32, tag="sq_q1")
    nc.scalar.activation(q1[:], sq[:], ACT.Identity, bias=1.0)
    den = smalls.tile([128, N], F32, tag="sq_den")
    nc.vector.tensor_mul(den[:], q1[:], t[:])
    rs = smalls.tile([128, N], F32, tag="sq_rs")
    nc.vector.reciprocal(rs[:], den[:])
    scale = smalls.tile([128, N], F32, tag="sq_scale")
    nc.vector.tensor_mul(scale[:], sq[:], rs[:])
    vb = vbpool.tile([128, ND], F32, tag="sq_vb")
    nc.vector.tensor_mul(
        vb[:].rearrange("p (n d) -> p n d", d=D),
        s4[:].rearrange("p (n d) -> p n d", d=D),
        scale[:, :, None].broadcast_to([128, N, D]),
    )
    return vb


def build_kernel():
    nc = bacc.Bacc("TRN2", target_bir_lowering=False, debug=False)

    xt = nc.dram_tensor("xt", [I_LOC * J, B], F32, kind="ExternalInput")
    wt = nc.dram_tensor("wt", [I_LOC * J, ND], F32, kind="ExternalInput")
    out = nc.dram_tensor("out", [B, N, D], F32, kind="ExternalOutput")

    # collective bounce buffers (one pair per round)
    s_in = [nc.dram_tensor(f"s_in{r}", [B, ND], F32) for r in range(R)]
    s_out = [nc.dram_tensor(f"s_out{r}", [B, ND], F32, addr_space="Shared")
             for r in range(R)]

    with tile.TileContext(nc) as tc:
        with (
            tc.tile_pool(name="persist", bufs=1) as pp,
            tc.tile_pool(name="wsbp", bufs=3) as wsbp,
            tc.tile_pool(name="wgp", bufs=4) as wgp,
            tc.tile_pool(name="vbp", bufs=1) as vbp,
            tc.tile_pool(name="work", bufs=3) as kp,
            tc.tile_pool(name="s4p", bufs=1) as s4p,
            tc.tile_pool(name="pbig", bufs=1) as pbig,
            tc.tile_pool(name="small", bufs=3) as sp,
            tc.tile_pool(name="psum", bufs=2, space="PSUM") as psp,
        ):
            # ---- resident tiles ----
            # x chunks for round-1 fused einsum: [128=(8i,16j), 32 chunks, B]
            xsb = pp.tile([128, I_LOC * J // 128, B], F32, tag="xsb")
            nc.sync.dma_start(
                xsb[:], xt[:].rearrange("(k p) b -> p k b", p=128))
            # x for per-capsule matmuls: [16=j, I_LOC, B] (partitions 0-15)
            xa = pp.tile([16, I_LOC, B], F32, tag="xa")
            nc.sync.dma_start(xa[:], xt[:].rearrange("(i j) b -> j i b", j=J))

            # routing logits b: [128=(c,b), GROUPS, N]
            bstate = pp.tile([128, GROUPS, N], F32, tag="bstate")
            nc.gpsimd.memset(bstate[:], 0.0)
            eps_t = pp.tile([128, 1], F32, tag="eps")
            nc.gpsimd.memset(eps_t[:], EPS)
            # partial s accumulator [128=(c,b), (n d)]
            s_acc = pp.tile([128, ND], F32, tag="s_acc")
            # selector[p, m] = 1.0 if p % 32 == m  (partition-group fold via PE)
            sel_i = pp.tile([128, B], mybir.dt.int32, tag="sel_i")
            nc.gpsimd.iota(sel_i[:], [[1, B]], channel_multiplier=-1)
            nc.vector.tensor_scalar(sel_i[:], sel_i[:], 31, None,
                                    op0=mybir.AluOpType.bitwise_and)
            sel = pp.tile([128, B], F32, tag="sel")
            nc.vector.tensor_scalar(sel[:], sel_i[:], 0, None,
                                    op0=mybir.AluOpType.is_equal)

            # ---------- round 0: c uniform -> s0 = (1/N) sum_i ihat ----------
            ps0 = psp.tile([B, ND], F32, tag="pg")
            n_chunks = I_LOC * J // 128  # 32
            for k in range(n_chunks):
                wsb = wsbp.tile([128, ND], F32, tag="wsb")
                nc.sync.dma_start(wsb[:], wt[k * 128:(k + 1) * 128, :])
                for q in range(ND // 512):
                    nc.tensor.matmul(
                        ps0[:, q * 512:(q + 1) * 512],
                        xsb[:, k, :],
                        wsb[:, q * 512:(q + 1) * 512],
                        start=(k == 0), stop=(k == n_chunks - 1),
                    )
            s_loc0 = pbig.tile([B, ND], F32, tag="s_loc")
            nc.scalar.mul(s_loc0[:], ps0[:], 1.0 / N)
            nc.sync.dma_start(s_in[0][:], s_loc0[:])
            nc.gpsimd.collective_compute(
                "AllReduce", ADD,
                replica_groups=[list(range(CORES))],
                ins=[s_in[0].ap().opt()], outs=[s_out[0].ap().opt()],
            )
            s4 = s4p.tile([128, ND], F32, tag="s4")
            for g4 in range(4):
                nc.sync.dma_start(s4[g4 * 32:(g4 + 1) * 32, :], s_out[0][:])
            vb = _squash_build(nc, kp, vbp, sp, s4, eps_t[:])

            # ---------- rounds 1, 2 ----------
            for r in (1, 2):
                nc.gpsimd.memset(s_acc[:], 0.0)
                for g in range(GROUPS):
                    # W rows for capsules i = 4g..4g+3 -> [16=j, 4, ND]
                    wgs = []
                    for c in range(4):
                        wgc = wgp.tile([16, ND], F32, tag="wg")
                        nc.sync.dma_start(
                            wgc[:],
                            wt[:].rearrange("(i j) f -> j i f", j=J)
                            [:, 4 * g + c, :],
                        )
                        wgs.append(wgc)
                    pg = psp.tile([128, ND], F32, tag="pg")
                    for c in range(4):
                        for q in range(ND // 512):
                            nc.tensor.matmul(
                                pg[32 * c:32 * (c + 1), q * 512:(q + 1) * 512],
                                xa[:, 4 * g + c, :],
                                wgs[c][:, q * 512:(q + 1) * 512],
                                start=True, stop=True,
                                tile_position=(0, 32 * c),
                            )
                    # y = sum_d H * v
                    tmp = kp.tile([128, ND], F32, tag="tmp")
                    nc.vector.tensor_mul(tmp[:], pg[:], vb[:])
                    y = sp.tile([128, N], F32, tag="y")
                    nc.vector.tensor_reduce(
                        y[:], tmp[:].rearrange("p (n d) -> p n d", d=D),
                        axis=FX, op=ADD)
                    # b += y
                    bsl = bstate[:, g, :]
                    nc.vector.tensor_add(bsl, bsl, y[:])
                    # c = softmax_n(b)
                    mx = sp.tile([128, 1], F32, tag="mx")
                    nc.vector.tensor_reduce(mx[:], bsl, axis=FX, op=MAX,
                                            negate=True)
                    e = sp.tile([128, N], F32, tag="e")
                    se = sp.tile([128, 1], F32, tag="se")
                    nc.scalar.activation(e[:], bsl, ACT.Exp, bias=mx[:],
                                         accum_out=se[:])
                    rcp = sp.tile([128, 1], F32, tag="rcp")
                    nc.vector.reciprocal(rcp[:], se[:])
                    cg = sp.tile([128, N], F32, tag="cg")
                    nc.vector.tensor_scalar_mul(cg[:], e[:], rcp[:])
                    # s_acc += c * H
                    tmp2 = kp.tile([128, ND], F32, tag="tmp")
                    nc.vector.tensor_mul(
                        tmp2[:].rearrange("p (n d) -> p n d", d=D),
                        pg[:].rearrange("p (n d) -> p n d", d=D),
                        cg[:, :, None].broadcast_to([128, N, D]),
                    )
                    nc.vector.tensor_add(s_acc[:], s_acc[:], tmp2[:])

                # fold the 4 partition groups via PE and AllReduce
                ps_f = psp.tile([B, ND], F32, tag="pg")
                for q in range(ND // 512):
                    nc.tensor.matmul(
                        ps_f[:, q * 512:(q + 1) * 512],
                        sel[:],
                        s_acc[:, q * 512:(q + 1) * 512],
                        start=True, stop=True,
                    )
                s_loc = pbig.tile([B, ND], F32, tag="s_loc")
                nc.scalar.copy(s_loc[:], ps_f[:])
                nc.sync.dma_start(s_in[r][:], s_loc[:])
                nc.gpsimd.collective_compute(
                    "AllReduce", ADD,
                    replica_groups=[list(range(CORES))],
                    ins=[s_in[r].ap().opt()], outs=[s_out[r].ap().opt()],
                )
                s4 = s4p.tile([128, ND], F32, tag="s4")
                for g4 in range(4):
                    nc.sync.dma_start(s4[g4 * 32:(g4 + 1) * 32, :],
                                      s_out[r][:])
                vb = _squash_build(nc, kp, vbp, sp, s4, eps_t[:])

            # output = squash(s2) = vb rows 0..31
            nc.sync.dma_start(
                out[:].rearrange("b n d -> b (n d)"), vb[0:32, :])

    nc.compile()
    return nc


_NC_CACHE = {}


def _get_nc():
    if "nc" not in _NC_CACHE:
        _NC_CACHE["nc"] = build_kernel()
    return _NC_CACHE["nc"]


def _make_in_maps(inputs, W):
    inputs = np.ascontiguousarray(np.asarray(inputs, dtype=np.float32))
    W = np.ascontiguousarray(np.asarray(W, dtype=np.float32))
    assert inputs.shape == (B, I, J) and W.shape == (N, I, D, J)
    in_maps = []
    for c in range(CORES):
        sl = slice(c * I_LOC, (c + 1) * I_LOC)
        # xt: [(i j), b]
        x_t = np.ascontiguousarray(
            inputs[:, sl, :].transpose(1, 2, 0).reshape(I_LOC * J, B))
        # wt: [(i j), (n d)] ; wt[(i,j),(n,d)] = W[n, i, d, j]
        w_t = np.ascontiguousarray(
            W[:, sl, :, :].transpose(1, 3, 0, 2).reshape(I_LOC * J, ND))
        in_maps.append({"xt": x_t, "wt": w_t})
    return in_maps


def _ensure_ntff_hook():
    """Register the axon NTFF profile hook if the image's antenv lacks it."""
    import types

    try:
        import antenv.axon_hooks  # noqa: F401
        return
    except ImportError:
        pass
    import antenv

    if "/root/.axon_site" not in sys.path:
        sys.path.insert(0, "/root/.axon_site")
    from trn_agent_boot.trn_boot import _ntff_profile_via_ctypes

    hook = {"h": _ntff_profile_via_ctypes("/opt/axon/libaxon_pjrt.so")}
    mod = types.ModuleType("antenv.axon_hooks")
    mod.get_axon_ntff_profile_hook = lambda: hook["h"]
    mod.set_axon_ntff_profile_hook = lambda h: hook.__setitem__("h", h)
    sys.modules["antenv.axon_hooks"] = mod
    antenv.axon_hooks = mod


def run(inputs, W, trace=False):
    nc = _get_nc()
    if trace:
        _ensure_ntff_hook()
        # zero-egress container: skip the artifact upload, keep files local
        import concourse.bass_utils as bu
        bu.upload_artifacts = lambda d: d
    res = run_bass_kernel_spmd(
        nc, _make_in_maps(inputs, W), core_ids=list(range(CORES)),
        trace=trace,
    )
    return res.results[0]["out"].reshape(B, N, D), res


def kernel(inputs, W, routings=R, **_unused):
    assert int(routings) == R
    out, _ = run(inputs, W, trace=False)
    return out


# revision 13
# speedup vs baseline: 1.2639x; 1.2639x over previous
"""CapsuleLayer dynamic-routing kernel for Trainium2 (8 NeuronCores).

Problem: inputs [B=32, I=2048, J=16], W [N=64, I=2048, D=32, J=16], routings=3.
  inputs_hat[b,n,i,d] = sum_j inputs[b,i,j] * W[n,i,d,j]
  3 rounds of routing (softmax over n, weighted sum over i, squash over d).

Strategy: shard the input-capsule axis I across the 8 cores (I_loc=256).
Each core recomputes its ihat shard from W each round (W streamed from HBM
as bf16 hi/lo pairs; ihat never hits DRAM), keeps its b-state [*, n, i_loc]
in SBUF, and the only cross-core data is the [B, N, D] partial sum s,
AllReduced (256 KB) once per round. Output replicated; host takes core 0's.

Matmuls run in bf16 with error compensation: x = xh + xl, W = Wh + Wl
(each bf16); rounds 1-2 accumulate xh*Wh + xh*Wl + xl*Wh in fp32 PSUM
(error ~2^-18). Round 0 uses xh*Wh only (it feeds logits, not the output).

On-chip layout per round, per group of 4 input capsules i:
  PE: col-tiled matmuls (tile_position=(0,32c)), K=j=16, M=b=32,
      Nf=(n,d)=2048 -> PSUM H-tile [128=(4i,32b), 2048=(64n,32d)]
  DVE/ACT: y = sum_d H*v ; b += y ; c = softmax_n(b) ; tmp2 = c*H
  PE: s_psum += selector.T @ tmp2  (folds partition groups AND sums over i)
"""

import sys

for p in ("/opt/trn_rl_repo",):
    if p not in sys.path:
        sys.path.insert(0, p)

import ml_dtypes
import numpy as np

import concourse.bacc as bacc
import concourse.mybir as mybir
import concourse.tile as tile
from concourse.bass_utils import run_bass_kernel_spmd

# problem constants (hardcoded per harness contract)
B, N, I, D, J = 32, 64, 2048, 32, 16
R = 3  # routings
CORES = 8
I_LOC = I // CORES  # 256
ND = N * D  # 2048
EPS = 1e-7

F32 = mybir.dt.float32
BF16 = mybir.dt.bfloat16
FX = mybir.AxisListType.X
ADD = mybir.AluOpType.add
MAX = mybir.AluOpType.max
ACT = mybir.ActivationFunctionType

GROUPS = I_LOC // 4  # 64 groups of 4 capsules per round
NQ = ND // 512  # free-dim quarters per capsule


def _squash_build(nc, vbpool, smalls, s4, eps_ap):
    """s4: [128, 2048] tile holding s (replicated x4 on partition groups).
    Returns vb [128, 2048] = squash(s) broadcast tile (same replication)."""
    s2 = vbpool.tile([128, ND], F32, tag="sq_s2")
    nc.vector.tensor_mul(s2[:], s4[:], s4[:])
    sq = smalls.tile([128, N], F32, tag="sq_sq")
    nc.vector.tensor_reduce(sq[:], s2[:].rearrange("p (n d) -> p n d", d=D),
                            axis=FX, op=ADD)
    # t = sqrt(sq + eps)
    t = smalls.tile([128, N], F32, tag="sq_t")
    nc.scalar.activation(t[:], sq[:], ACT.Sqrt, bias=eps_ap)
    # q1 = 1 + sq
    q1 = smalls.tile([128, N], F32, tag="sq_q1")
    nc.scalar.activation(q1[:], sq[:], ACT.Identity, bias=1.0)
    den = smalls.tile([128, N], F32, tag="sq_den")
    nc.vector.tensor_mul(den[:], q1[:], t[:])
    rs = smalls.tile([128, N], F32, tag="sq_rs")
    nc.vector.reciprocal(rs[:], den[:])
    scale = smalls.tile([128, N], F32, tag="sq_scale")
    nc.vector.tensor_mul(scale[:], sq[:], rs[:])
    vb = vbpool.tile([128, ND], F32, tag="sq_vb")
    nc.vector.tensor_mul(
        vb[:].rearrange("p (n d) -> p n d", d=D),
        s4[:].rearrange("p (n d) -> p n d", d=D),
        scale[:, :, None].broadcast_to([128, N, D]),
    )
    return vb


def build_kernel():
    nc = bacc.Bacc("TRN2", target_bir_lowering=False, debug=False)

    xth = nc.dram_tensor("xth", [I_LOC * J, B], BF16, kind="ExternalInput")
    xtl = nc.dram_tensor("xtl", [I_LOC * J, B], BF16, kind="ExternalInput")
    wth = nc.dram_tensor("wth", [I_LOC * J, ND], BF16, kind="ExternalInput")
    wtl = nc.dram_tensor("wtl", [I_LOC * J, ND], BF16, kind="ExternalInput")
    out = nc.dram_tensor("out", [B, N, D], F32, kind="ExternalOutput")

    # collective bounce buffers (one pair per round)
    s_in = [nc.dram_tensor(f"s_in{r}", [B, ND], F32) for r in range(R)]
    s_out = [nc.dram_tensor(f"s_out{r}", [B, ND], F32, addr_space="Shared")
             for r in range(R)]

    wth_v = wth[:].rearrange("(i j) f -> j i f", j=J)
    wtl_v = wtl[:].rearrange("(i j) f -> j i f", j=J)

    with tile.TileContext(nc) as tc:
        with (
            tc.tile_pool(name="persist", bufs=1) as pp,
            tc.tile_pool(name="wsbp", bufs=3) as wsbp,
            tc.tile_pool(name="wgp", bufs=4) as wgp,
            tc.tile_pool(name="vbp", bufs=1) as vbp,
            tc.tile_pool(name="work", bufs=3) as kp,
            tc.tile_pool(name="s4p", bufs=1) as s4p,
            tc.tile_pool(name="pbig", bufs=1) as pbig,
            tc.tile_pool(name="small", bufs=3) as sp,
            tc.tile_pool(name="psum", bufs=2, space="PSUM") as psp,
            tc.tile_pool(name="psumB", bufs=1, space="PSUM") as psB,
        ):
            # ---- resident tiles ----
            # x chunks for round-0 fused einsum: [128=(8i,16j), 32 chunks, B]
            xsbh = pp.tile([128, I_LOC * J // 128, B], BF16, tag="xsbh")
            nc.sync.dma_start(
                xsbh[:], xth[:].rearrange("(k p) b -> p k b", p=128))
            # x for per-capsule matmuls: [16=j, I_LOC, B] (partitions 0-15)
            xah = pp.tile([16, I_LOC, B], BF16, tag="xah")
            nc.sync.dma_start(xah[:],
                              xth[:].rearrange("(i j) b -> j i b", j=J))
            xal = pp.tile([16, I_LOC, B], BF16, tag="xal")
            nc.sync.dma_start(xal[:],
                              xtl[:].rearrange("(i j) b -> j i b", j=J))

            # routing logits b: [128=(c,b), GROUPS, N]
            bstate = pp.tile([128, GROUPS, N], F32, tag="bstate")
            nc.gpsimd.memset(bstate[:], 0.0)
            eps_t = pp.tile([128, 1], F32, tag="eps")
            nc.gpsimd.memset(eps_t[:], EPS)
            # selector[p, m] = 1.0 if p % 32 == m  (partition-group fold)
            sel_i = pp.tile([128, B], mybir.dt.int32, tag="sel_i")
            nc.gpsimd.iota(sel_i[:], [[1, B]], channel_multiplier=-1)
            nc.vector.tensor_scalar(sel_i[:], sel_i[:], 31, None,
                                    op0=mybir.AluOpType.bitwise_and)
            sel = pp.tile([128, B], F32, tag="sel")
            nc.vector.tensor_scalar(sel[:], sel_i[:], 0, None,
                                    op0=mybir.AluOpType.is_equal)

            # ---------- round 0: c uniform -> s0 = (1/N) sum_i ihat ----------
            ps0 = psB.tile([B, ND], F32, tag="pss")
            n_chunks = I_LOC * J // 128  # 32
            for k in range(n_chunks):
                wsb = wsbp.tile([128, ND], BF16, tag="wsb")
                nc.sync.dma_start(wsb[:], wth[k * 128:(k + 1) * 128, :])
                for q in range(NQ):
                    nc.tensor.matmul(
                        ps0[:, q * 512:(q + 1) * 512],
                        xsbh[:, k, :],
                        wsb[:, q * 512:(q + 1) * 512],
                        start=(k == 0), stop=(k == n_chunks - 1),
                    )
            s_loc0 = pbig.tile([B, ND], F32, tag="s_loc")
            nc.scalar.mul(s_loc0[:], ps0[:], 1.0 / N)
            nc.sync.dma_start(s_in[0][:], s_loc0[:])
            nc.gpsimd.collective_compute(
                "AllReduce", ADD,
                replica_groups=[list(range(CORES))],
                ins=[s_in[0].ap().opt()], outs=[s_out[0].ap().opt()],
            )
            s4 = s4p.tile([128, ND], F32, tag="s4")
            for g4 in range(4):
                nc.sync.dma_start(s4[g4 * 32:(g4 + 1) * 32, :], s_out[0][:])
            vb = _squash_build(nc, vbp, sp, s4, eps_t[:])

            # ---------- rounds 1, 2 ----------
            HF = ND // 2  # 1024: free-dim half (n 0-31 / n 32-63)
            for r in (1, 2):
                ps_s = psB.tile([B, ND], F32, tag="pss")
                for g in range(GROUPS):
                    # W rows for capsules i = 4g..4g+3 -> [16=j, 4, ND]
                    wgh = wgp.tile([16, 4, ND], BF16, tag="wg")
                    nc.sync.dma_start(wgh[:], wth_v[:, 4 * g:4 * g + 4, :])
                    wgl = wgp.tile([16, 4, ND], BF16, tag="wg")
                    nc.sync.dma_start(wgl[:], wtl_v[:, 4 * g:4 * g + 4, :])
                    y = sp.tile([128, N], F32, tag="y")
                    hsbs = []
                    for h in range(2):
                        pg = psp.tile([128, HF], F32, tag="pg")
                        for c in range(4):
                            i = 4 * g + c
                            prods = [(xah, wgh), (xah, wgl), (xal, wgh)]
                            nmm = len(prods) * 2
                            m = 0
                            for xa_, wg_ in prods:
                                for q in range(2):
                                    f0 = h * HF + q * 512
                                    nc.tensor.matmul(
                                        pg[32 * c:32 * (c + 1),
                                           q * 512:(q + 1) * 512],
                                        xa_[:, i, :],
                                        wg_[:, c, f0:f0 + 512],
                                        start=(m < 2), stop=(m >= nmm - 2),
                                        tile_position=(0, 32 * c),
                                    )
                                    m += 1
                        # free PSUM fast: stage H half to SBUF on ScalarE
                        hsb = kp.tile([128, HF], F32, tag="hsb")
                        nc.scalar.copy(hsb[:], pg[:])
                        hsbs.append(hsb)
                        # y[, n-half] = sum_d H * v
                        tmp = kp.tile([128, HF], F32, tag="tmp")
                        nc.vector.tensor_mul(tmp[:], pg[:],
                                             vb[:, h * HF:(h + 1) * HF])
                        nc.vector.tensor_reduce(
                            y[:, 32 * h:32 * (h + 1)],
                            tmp[:].rearrange("p (n d) -> p n d", d=D),
                            axis=FX, op=ADD)
                    # b += y
                    bsl = bstate[:, g, :]
                    nc.vector.tensor_add(bsl, bsl, y[:])
                    # c = softmax_n(b)
                    mx = sp.tile([128, 1], F32, tag="mx")
                    nc.vector.tensor_reduce(mx[:], bsl, axis=FX, op=MAX,
                                            negate=True)
                    e = sp.tile([128, N], F32, tag="e")
                    se = sp.tile([128, 1], F32, tag="se")
                    nc.scalar.activation(e[:], bsl, ACT.Exp, bias=mx[:],
                                         accum_out=se[:])
                    rcp = sp.tile([128, 1], F32, tag="rcp")
                    nc.vector.reciprocal(rcp[:], se[:])
                    cg = sp.tile([128, N], F32, tag="cg")
                    nc.vector.tensor_scalar_mul(cg[:], e[:], rcp[:])
                    # tmp2 = c * H ; s_psum += selector.T @ tmp2
                    for h in range(2):
                        tmp2 = kp.tile([128, HF], F32, tag="tmp")
                        eng = nc.vector if h == 0 else nc.gpsimd
                        eng.tensor_mul(
                            tmp2[:].rearrange("p (n d) -> p n d", d=D),
                            hsbs[h][:].rearrange("p (n d) -> p n d", d=D),
                            cg[:, 32 * h:32 * (h + 1), None].broadcast_to(
                                [128, 32, D]),
                        )
                        for q in range(2):
                            f0 = h * HF + q * 512
                            nc.tensor.matmul(
                                ps_s[:, f0:f0 + 512],
                                sel[:],
                                tmp2[:, q * 512:(q + 1) * 512],
                                start=(g == 0), stop=(g == GROUPS - 1),
                                skip_group_check=True,
                            )

                s_loc = pbig.tile([B, ND], F32, tag="s_loc")
                nc.scalar.copy(s_loc[:], ps_s[:])
                nc.sync.dma_start(s_in[r][:], s_loc[:])
                nc.gpsimd.collective_compute(
                    "AllReduce", ADD,
                    replica_groups=[list(range(CORES))],
                    ins=[s_in[r].ap().opt()], outs=[s_out[r].ap().opt()],
                )
                s4 = s4p.tile([128, ND], F32, tag="s4")
                for g4 in range(4):
                    nc.sync.dma_start(s4[g4 * 32:(g4 + 1) * 32, :],
                                      s_out[r][:])
                vb = _squash_build(nc, vbp, sp, s4, eps_t[:])

            # output = squash(s2) = vb rows 0..31
            nc.sync.dma_start(
                out[:].rearrange("b n d -> b (n d)"), vb[0:32, :])

    nc.compile()
    return nc


_NC_CACHE = {}


def _get_nc():
    if "nc" not in _NC_CACHE:
        _NC_CACHE["nc"] = build_kernel()
    return _NC_CACHE["nc"]


def _hi_lo(a):
    hi = a.astype(ml_dtypes.bfloat16)
    lo = (a - hi.astype(np.float32)).astype(ml_dtypes.bfloat16)
    return hi, lo


def _make_in_maps(inputs, W):
    inputs = np.ascontiguousarray(np.asarray(inputs, dtype=np.float32))
    W = np.ascontiguousarray(np.asarray(W, dtype=np.float32))
    assert inputs.shape == (B, I, J) and W.shape == (N, I, D, J)
    in_maps = []
    for c in range(CORES):
        sl = slice(c * I_LOC, (c + 1) * I_LOC)
        # xt: [(i j), b]
        x_t = np.ascontiguousarray(
            inputs[:, sl, :].transpose(1, 2, 0).reshape(I_LOC * J, B))
        # wt: [(i j), (n d)] ; wt[(i,j),(n,d)] = W[n, i, d, j]
        w_t = np.ascontiguousarray(
            W[:, sl, :, :].transpose(1, 3, 0, 2).reshape(I_LOC * J, ND))
        xh, xl = _hi_lo(x_t)
        wh, wl = _hi_lo(w_t)
        in_maps.append({"xth": np.ascontiguousarray(xh),
                        "xtl": np.ascontiguousarray(xl),
                        "wth": np.ascontiguousarray(wh),
                        "wtl": np.ascontiguousarray(wl)})
    return in_maps


def _ensure_ntff_hook():
    """Register the axon NTFF profile hook if the image's antenv lacks it."""
    import types

    try:
        import antenv.axon_hooks  # noqa: F401
        return
    except ImportError:
        pass
    import antenv

    if "/root/.axon_site" not in sys.path:
        sys.path.insert(0, "/root/.axon_site")
    from trn_agent_boot.trn_boot import _ntff_profile_via_ctypes

    hook = {"h": _ntff_profile_via_ctypes("/opt/axon/libaxon_pjrt.so")}
    mod = types.ModuleType("antenv.axon_hooks")
    mod.get_axon_ntff_profile_hook = lambda: hook["h"]
    mod.set_axon_ntff_profile_hook = lambda h: hook.__setitem__("h", h)
    sys.modules["antenv.axon_hooks"] = mod
    antenv.axon_hooks = mod


def run(inputs, W, trace=False):
    nc = _get_nc()
    if trace:
        _ensure_ntff_hook()
        # zero-egress container: skip the artifact upload, keep files local
        import concourse.bass_utils as bu
        bu.upload_artifacts = lambda d: d
    res = run_bass_kernel_spmd(
        nc, _make_in_maps(inputs, W), core_ids=list(range(CORES)),
        trace=trace,
    )
    return res.results[0]["out"].reshape(B, N, D), res


def kernel(inputs, W, routings=R, **_unused):
    assert int(routings) == R
    out, _ = run(inputs, W, trace=False)
    return out
